# revision 1
# baseline (speedup 1.0000x reference)
"""Chamfer loss kernel for Trainium2 (8 NeuronCores, data-parallel over batch).

Problem: x [32, 2048, 3], y [32, 2048, 3] fp32.
  dist[b, m, n] = ||x[b, n] - y[b, m]||^2
  row[b] = mean_n min_m dist ; col[b] = mean_m min_n dist
  out = mean_b max(row, col)

Per core (4 batches): dist[m, n] = yfeat[:, m] . xfeat[:, n], K=5 features
  yfeat = [y0, y1, y2, ||y||^2, 1], xfeat = [-2*x0, -2*x1, -2*x2, 1, ||x||^2]
PE -> PSUM [128 m x 512 n] fp32 tiles; DVE reduces:
  colmin (min over n, free axis) via tensor_reduce per tile,
  rowacc (min over m-chunks, elementwise) via tensor_tensor min into fp16.
Host: rowmin[n] = min_p rowacc[p, n]; means; max; mean over batch.
"""

import os
import sys

import numpy as np

if "/opt/trn_rl_repo" not in sys.path:
    sys.path.insert(0, "/opt/trn_rl_repo")

B, N, M, D = 32, 2048, 2048, 3
N_CORES = 8
BPC = B // N_CORES  # batches per core = 4
MCH = 16  # m-chunks of 128
NCH = 4  # n-chunks of 512

_CACHE = {}
LAST_RESULTS = None


def _build_bass(repeats=1):
    import concourse.bass as bass
    import concourse.tile as tile
    from concourse import mybir

    F32 = mybir.dt.float32
    F16 = mybir.dt.float16
    MIN = mybir.AluOpType.min

    nc = bass.Bass()
    # feats[0] = xfeat [BPC, 5, N], feats[1] = yfeat [BPC, 5, M]
    feats = nc.dram_tensor("feats", [2, BPC, 5, N], F32, kind="ExternalInput")
    # out16[:, : BPC*N]   = rowacc: [p, b, n] -> min over i of dist[128*i+p, n]
    # out16[:, BPC*N :]   = colmin: [p, b*16+i] -> min over n of dist[128*i+p, n]
    out16 = nc.dram_tensor(
        "out16", [128, BPC * N + BPC * MCH], F16, kind="ExternalOutput"
    )

    with tile.TileContext(nc) as tc:
        with (
            tc.tile_pool(name="feat", bufs=1) as featp,
            tc.tile_pool(name="psum", bufs=8, space="PSUM") as psump,
            tc.tile_pool(name="acc", bufs=1) as accp,
            tc.tile_pool(name="colp", bufs=4) as colpp,
        ):
            ft = featp.tile([5, 2 * BPC, N], F32, tag="ft")
            in_dma = nc.sync.dma_start(
                out=ft[:], in_=feats[:].rearrange("t b k n -> k (t b) n")
            )
            xft = ft[:, 0:BPC, :]
            yft = ft[:, BPC : 2 * BPC, :]

            packed = accp.tile([128, BPC * N + BPC * MCH], F16, tag="packed")
            racc = packed[:, 0 : BPC * N].rearrange("p (b n) -> p b n", b=BPC)
            colsb = packed[:, BPC * N :]

            last_mm = None
            for _r in range(repeats):
              for b in range(BPC):
                for i in range(MCH):
                    colpart = colpp.tile([128, NCH], F32, tag="colpart")
                    for j in range(NCH):
                        ps = psump.tile([128, 512], F32, tag="ps")
                        last_mm = nc.tensor.matmul(
                            ps[:],
                            yft[:, b, 128 * i : 128 * (i + 1)],
                            xft[:, b, 512 * j : 512 * (j + 1)],
                            start=True,
                            stop=True,
                        )
                        nc.vector.tensor_reduce(
                            out=colpart[:, j : j + 1],
                            in_=ps[:],
                            axis=mybir.AxisListType.X,
                            op=MIN,
                        )
                        if i == 0:
                            nc.vector.tensor_copy(
                                racc[:, b, 512 * j : 512 * (j + 1)], ps[:]
                            )
                        else:
                            nc.vector.tensor_tensor(
                                racc[:, b, 512 * j : 512 * (j + 1)],
                                ps[:],
                                racc[:, b, 512 * j : 512 * (j + 1)],
                                MIN,
                            )
                    last_dve = nc.vector.tensor_reduce(
                        out=colsb[:, b * MCH + i : b * MCH + i + 1],
                        in_=colpart[:],
                        axis=mybir.AxisListType.X,
                        op=MIN,
                    )
            # Pre-observe PE and the input-DMA lane on the SP engine so the
            # Tile end-of-kernel Drain needs only 2 sem waits (DVE + out-DMA);
            # walrus rejects instructions with >2 sync waits.
            from concourse.tile_rust import add_dep_helper

            nop1 = nc.sync.nop(nofuse=True)
            add_dep_helper(nop1.ins, last_mm.ins, sync=True, reason="observe PE")
            nop2 = nc.sync.nop(nofuse=True)
            add_dep_helper(nop2.ins, in_dma.ins, sync=True, reason="observe in-dma")
            nc.sync.dma_start(out=out16[:], in_=packed[:])

    # The Tile end-of-kernel SP Drain waits on every outstanding proc, but
    # walrus only allows 1 sync wait on a Drain. Engine waits (PE/DVE) are
    # redundant with the all-engine barrier that follows the drain (each
    # engine's barrier inc is program-ordered after its last op) — keep only
    # DMA-lane waits, which the barrier does not cover.
    for fn in nc.m.functions:
        for bb in fn.blocks:
            for ins in bb.instructions:
                if ins.__class__.__name__ == "InstDrain" and ins.sync_info is not None:
                    w = ins.sync_info.on_wait
                    if len(w) > 1:
                        keep = [x for x in w if x.ant_name.startswith("DMA")]
                        assert len(keep) <= 1, [x.ant_name for x in w]
                        ins.sync_info.on_wait = keep

    return nc


def _prep_core_inputs(x, y, c):
    xb = x[BPC * c : BPC * (c + 1)]  # [4, 2048, 3]
    yb = y[BPC * c : BPC * (c + 1)]
    ones = np.ones((BPC, N), np.float32)
    x2 = np.sum(xb.astype(np.float32) ** 2, axis=-1)  # [4, N]
    y2 = np.sum(yb.astype(np.float32) ** 2, axis=-1)  # [4, M]
    xfeat = np.stack(
        [-2.0 * xb[..., 0], -2.0 * xb[..., 1], -2.0 * xb[..., 2], ones, x2], axis=1
    ).astype(np.float32)  # [4, 5, N]
    yfeat = np.stack(
        [yb[..., 0], yb[..., 1], yb[..., 2], y2, ones], axis=1
    ).astype(np.float32)  # [4, 5, M]
    return np.ascontiguousarray(np.stack([xfeat, yfeat], axis=0))  # [2, 4, 5, N]


def kernel(x, y):
    global LAST_RESULTS
    from concourse.bass_utils import run_bass_kernel_spmd

    x = np.asarray(x, dtype=np.float32)
    y = np.asarray(y, dtype=np.float32)
    assert x.shape == (B, N, D) and y.shape == (B, M, D)

    if "nc" not in _CACHE:
        _CACHE["nc"] = _build_bass()
    nc = _CACHE["nc"]

    in_maps = []
    for c in range(N_CORES):
        in_maps.append({"feats": _prep_core_inputs(x, y, c)})

    res = run_bass_kernel_spmd(nc, in_maps, core_ids=list(range(N_CORES)))
    LAST_RESULTS = res

    cham = np.zeros((B,), np.float64)
    for c in range(N_CORES):
        out = res.results[c]["out16"]  # [128, BPC*N + BPC*MCH] fp16
        rowacc = out[:, : BPC * N].reshape(128, BPC, N)
        colmin = out[:, BPC * N :].reshape(128, BPC, MCH)
        rowmin = rowacc.min(axis=0).astype(np.float64)  # [4, N]
        row = rowmin.mean(axis=1)  # [4]
        for b in range(BPC):
            col = colmin[:, b, :].astype(np.float64).mean()
            cham[BPC * c + b] = max(row[b], col)
    return np.float32(cham.mean())



# revision 8
# speedup vs baseline: 256.5228x; 256.5228x over previous
"""Chamfer loss kernel for Trainium2 (8 NeuronCores, data-parallel over batch).

Problem: x [32, 2048, 3], y [32, 2048, 3] fp32.
  dist[b, m, n] = ||x[b, n] - y[b, m]||^2
  row[b] = mean_n min_m dist ; col[b] = mean_m min_n dist
  out = mean_b max(row, col)

Per core (4 batches): dist[m, n] = yfeat[:, m] . xfeat[:, n] with K=15
compensated-fp16 features (hi/lo split so a single fp16-rate matmul carries
full fp32-grade precision; only the lo*lo term ~2^-22 is dropped):
  lhs rows = [Yh(5), Yh(5), Yl(5)], rhs rows = [Xh(5), Xl(5), Xh(5)]
  Y = [y0, y1, y2, ||y||^2, 1],  X = [-2*x0, -2*x1, -2*x2, 1, ||x||^2]
Structure per (b, i-of-16): 4 matmuls fill a [128 m x 2048 n] fp32 PSUM
supertile (4 banks); the scalar (ACT) engine casts it to an fp16 SBUF tile
(the only PSUM reader, freeing the bank early); DVE then runs both min ops
in fp16 2x mode: ONE min-reduce over the free axis (-> colmin for 128 m
rows) and ONE elementwise min into racc (row direction). The repeat loop
used for timing amplification is a For_i hardware loop so the NEFF stays
the same size for any repeat count (per-call NEFF load cancels in the
slope). Host: rowmin[n] = min_p racc[p, n]; means; max; mean over batch.
"""

import os
import sys

import numpy as np

if "/opt/trn_rl_repo" not in sys.path:
    sys.path.insert(0, "/opt/trn_rl_repo")

B, N, M, D = 32, 2048, 2048, 3
N_CORES = 8
BPC = B // N_CORES  # batches per core = 4
MCH = 16  # m-chunks of 128

_CACHE = {}
LAST_RESULTS = None


def _fix_walrus_waits(nc):
    """Walrus codegen rejects instructions with more than one sync wait
    (observed for Matmult, Drain, NoOp, and Pool TensorTensor). Hoist excess
    waits onto freshly inserted same-engine NoOps immediately before the
    instruction (engines are in-order, so waiting earlier is equivalent)."""
    from concourse import mybir

    def limit(opcode):
        return 1

    n_inserted = 0
    for fn in nc.m.functions:
        for bb in fn.blocks:
            new_instructions = []
            for ins in bb.instructions:
                si = getattr(ins, "sync_info", None)
                if si is not None and si.on_wait:
                    kept = list(si.on_wait)
                    while len(kept) > limit(ins.opcode):
                        w = kept.pop(0)
                        nop = mybir.InstNoOp(
                            name=f"{ins.name}-wfix{n_inserted}", ins=[], outs=[]
                        )
                        nop.engine = ins.engine
                        nop.sync_info = mybir.SyncInfo(on_wait=[w], on_update=[])
                        new_instructions.append(nop)
                        n_inserted += 1
                    si.on_wait = kept
                new_instructions.append(ins)
            bb.instructions[:] = new_instructions


def _build_bass(repeats=1):
    import concourse.bass as bass
    import concourse.tile as tile
    from concourse import mybir

    F32 = mybir.dt.float32
    F16 = mybir.dt.float16
    MIN = mybir.AluOpType.min
    COPY = mybir.ActivationFunctionType.Copy

    nc = bass.Bass()
    # feats[0] = xfeat (rhs) [BPC, 15, N], feats[1] = yfeat (lhs) [BPC, 15, M]
    feats = nc.dram_tensor("feats", [2, BPC, 15, N], F16, kind="ExternalInput")
    # out16[:, : BPC*N]   = racc: [p, b, n] -> min over i of dist[128*i+p, n]
    # out16[:, BPC*N :]   = colmin: [p, b*16+i] -> min over n of dist[128*i+p, n]
    out16 = nc.dram_tensor(
        "out16", [128, BPC * N + BPC * MCH], F16, kind="ExternalOutput"
    )

    with tile.TileContext(nc) as tc:
        with (
            tc.tile_pool(name="feat", bufs=1) as featp,
            tc.tile_pool(name="psum", bufs=2, space="PSUM") as psump,
            tc.tile_pool(name="acc", bufs=1) as accp,
            tc.tile_pool(name="s16p", bufs=4) as s16p,
        ):
            ft = featp.tile([15, 2 * BPC, N], F16, tag="ft")
            nc.sync.dma_start(
                out=ft[:], in_=feats[:].rearrange("t b k n -> k (t b) n")
            )
            xft = ft[:, 0:BPC, :]
            yft = ft[:, BPC : 2 * BPC, :]

            packed = accp.tile([128, BPC * N + BPC * MCH], F16, tag="packed")
            racc = packed[:, 0 : BPC * N].rearrange("p (b n) -> p b n", b=BPC)
            colsb = packed[:, BPC * N :]

            with tc.For_i(0, repeats) as _r:
                for b in range(BPC):
                    for i in range(MCH):
                        ps = psump.tile([128, 2048], F32, tag="ps")
                        for j in range(4):
                            nc.tensor.matmul(
                                ps[:, 512 * j : 512 * (j + 1)],
                                yft[:, b, 128 * i : 128 * (i + 1)],
                                xft[:, b, 512 * j : 512 * (j + 1)],
                                start=True,
                                stop=True,
                            )
                        s16 = s16p.tile([128, 2048], F16, tag="s16")
                        nc.scalar.activation(s16[:], ps[:], COPY)
                        nc.vector.tensor_reduce(
                            out=colsb[:, b * MCH + i : b * MCH + i + 1],
                            in_=s16[:],
                            axis=mybir.AxisListType.X,
                            op=MIN,
                        )
                        sl = racc[:, b, :]
                        if i == 0:
                            nc.vector.tensor_copy(sl, s16[:])
                        else:
                            nc.vector.tensor_tensor(sl, s16[:], sl, MIN)
            nc.sync.dma_start(out=out16[:], in_=packed[:])

    _fix_walrus_waits(nc)
    return nc


def _prep_core_inputs(x, y, c):
    """fp16 hi/lo compensated K=15 feature stacks for this core's batches."""
    xb = x[BPC * c : BPC * (c + 1)].astype(np.float32)  # [4, 2048, 3]
    yb = y[BPC * c : BPC * (c + 1)].astype(np.float32)
    ones = np.ones((BPC, N), np.float32)
    x2 = np.sum(xb**2, axis=-1)  # [4, N]
    y2 = np.sum(yb**2, axis=-1)  # [4, M]
    X = np.stack(
        [-2.0 * xb[..., 0], -2.0 * xb[..., 1], -2.0 * xb[..., 2], ones, x2], axis=1
    )  # [4, 5, N]
    Y = np.stack([yb[..., 0], yb[..., 1], yb[..., 2], y2, ones], axis=1)
    Xh = X.astype(np.float16)
    Xl = (X - Xh.astype(np.float32)).astype(np.float16)
    Yh = Y.astype(np.float16)
    Yl = (Y - Yh.astype(np.float32)).astype(np.float16)
    rhs = np.concatenate([Xh, Xl, Xh], axis=1)  # [4, 15, N]
    lhs = np.concatenate([Yh, Yh, Yl], axis=1)  # [4, 15, M]
    return np.ascontiguousarray(np.stack([rhs, lhs], axis=0))  # [2, 4, 15, N]


def kernel(x, y):
    global LAST_RESULTS
    from concourse.bass_utils import run_bass_kernel_spmd

    x = np.asarray(x, dtype=np.float32)
    y = np.asarray(y, dtype=np.float32)
    assert x.shape == (B, N, D) and y.shape == (B, M, D)

    if "nc" not in _CACHE:
        _CACHE["nc"] = _build_bass()
    nc = _CACHE["nc"]

    in_maps = []
    for c in range(N_CORES):
        in_maps.append({"feats": _prep_core_inputs(x, y, c)})

    res = run_bass_kernel_spmd(nc, in_maps, core_ids=list(range(N_CORES)))
    LAST_RESULTS = res

    cham = np.zeros((B,), np.float64)
    for c in range(N_CORES):
        out = res.results[c]["out16"]  # [128, BPC*N + BPC*MCH] fp16
        rowacc = out[:, : BPC * N].reshape(128, BPC, N)
        colmin = out[:, BPC * N :].reshape(128, BPC, MCH)
        rowmin = rowacc.min(axis=0).astype(np.float64)  # [4, N]
        row = rowmin.mean(axis=1)  # [4]
        for b in range(BPC):
            col = colmin[:, b, :].astype(np.float64).mean()
            cham[BPC * c + b] = max(row[b], col)
    return np.float32(cham.mean())


# revision 11
# speedup vs baseline: 278.9097x; 1.0873x over previous
"""Chamfer loss kernel for Trainium2 (8 NeuronCores, data-parallel over batch).

Problem: x [32, 2048, 3], y [32, 2048, 3] fp32.
  dist[b, m, n] = ||x[b, n] - y[b, m]||^2
  row[b] = mean_n min_m dist ; col[b] = mean_m min_n dist
  out = mean_b max(row, col)

Per core (4 batches): dist[m, n] = yfeat[:, m] . xfeat[:, n] with K=15
compensated-fp16 features (hi/lo split so a single fp16-rate matmul carries
full fp32-grade precision; only the lo*lo term ~2^-22 is dropped):
  lhs rows = [Yh(5), Yh(5), Yl(5)], rhs rows = [Xh(5), Xl(5), Xh(5)]
  Y = [y0, y1, y2, ||y||^2, 1],  X = [-2*x0, -2*x1, -2*x2, 1, ||x||^2]
Structure per (b, i-of-16): 4 matmuls fill a [128 m x 2048 n] fp32 PSUM
supertile (4 banks); the scalar (ACT) engine casts it to an fp16 SBUF tile
(the only PSUM reader, freeing the bank early); DVE then runs all min ops
in fp16 2x mode: two tensor_tensor folds (2048->1024->512, TT gets the 2x
perf mode that TensorReduce lacks) + a short 1x min-reduce over 512
(-> colmin for 128 m rows), and ONE elementwise min into racc (row
direction). The repeat loop
used for timing amplification is a For_i hardware loop so the NEFF stays
the same size for any repeat count (per-call NEFF load cancels in the
slope). Host: rowmin[n] = min_p racc[p, n]; means; max; mean over batch.
"""

import os
import sys

import numpy as np

if "/opt/trn_rl_repo" not in sys.path:
    sys.path.insert(0, "/opt/trn_rl_repo")

B, N, M, D = 32, 2048, 2048, 3
N_CORES = 8
BPC = B // N_CORES  # batches per core = 4
MCH = 16  # m-chunks of 128

_CACHE = {}
LAST_RESULTS = None


def _fix_walrus_waits(nc):
    """Walrus codegen rejects instructions with more than one sync wait
    (observed for Matmult, Drain, NoOp, and Pool TensorTensor). Hoist excess
    waits onto freshly inserted same-engine NoOps immediately before the
    instruction (engines are in-order, so waiting earlier is equivalent)."""
    from concourse import mybir

    def limit(opcode):
        return 1

    n_inserted = 0
    for fn in nc.m.functions:
        for bb in fn.blocks:
            new_instructions = []
            for ins in bb.instructions:
                si = getattr(ins, "sync_info", None)
                if si is not None and si.on_wait:
                    kept = list(si.on_wait)
                    while len(kept) > limit(ins.opcode):
                        w = kept.pop(0)
                        nop = mybir.InstNoOp(
                            name=f"{ins.name}-wfix{n_inserted}", ins=[], outs=[]
                        )
                        nop.engine = ins.engine
                        nop.sync_info = mybir.SyncInfo(on_wait=[w], on_update=[])
                        new_instructions.append(nop)
                        n_inserted += 1
                    si.on_wait = kept
                new_instructions.append(ins)
            bb.instructions[:] = new_instructions


def _build_bass(repeats=1):
    import concourse.bass as bass
    import concourse.tile as tile
    from concourse import mybir

    F32 = mybir.dt.float32
    F16 = mybir.dt.float16
    MIN = mybir.AluOpType.min
    COPY = mybir.ActivationFunctionType.Copy

    nc = bass.Bass()
    # feats[0] = xfeat (rhs) [BPC, 15, N], feats[1] = yfeat (lhs) [BPC, 15, M]
    feats = nc.dram_tensor("feats", [2, BPC, 15, N], F16, kind="ExternalInput")
    # out16[:, : BPC*N]   = racc: [p, b, n] -> min over i of dist[128*i+p, n]
    # out16[:, BPC*N :]   = colmin: [p, b*16+i] -> min over n of dist[128*i+p, n]
    out16 = nc.dram_tensor(
        "out16", [128, BPC * N + BPC * MCH], F16, kind="ExternalOutput"
    )

    with tile.TileContext(nc) as tc:
        with (
            tc.tile_pool(name="feat", bufs=1) as featp,
            tc.tile_pool(name="psum", bufs=2, space="PSUM") as psump,
            tc.tile_pool(name="acc", bufs=1) as accp,
            tc.tile_pool(name="s16p", bufs=4) as s16p,
            tc.tile_pool(name="foldp", bufs=2) as foldp,
        ):
            ft = featp.tile([15, 2 * BPC, N], F16, tag="ft")
            nc.sync.dma_start(
                out=ft[:], in_=feats[:].rearrange("t b k n -> k (t b) n")
            )
            xft = ft[:, 0:BPC, :]
            yft = ft[:, BPC : 2 * BPC, :]

            packed = accp.tile([128, BPC * N + BPC * MCH], F16, tag="packed")
            racc = packed[:, 0 : BPC * N].rearrange("p (b n) -> p b n", b=BPC)
            colsb = packed[:, BPC * N :]

            with tc.For_i(0, repeats) as _r:
                for b in range(BPC):
                    for i in range(MCH):
                        ps = psump.tile([128, 2048], F32, tag="ps")
                        for j in range(4):
                            nc.tensor.matmul(
                                ps[:, 512 * j : 512 * (j + 1)],
                                yft[:, b, 128 * i : 128 * (i + 1)],
                                xft[:, b, 512 * j : 512 * (j + 1)],
                                start=True,
                                stop=True,
                            )
                        s16 = s16p.tile([128, 2048], F16, tag="s16")
                        nc.scalar.activation(s16[:], ps[:], COPY)
                        fold = foldp.tile([128, 1024], F16, tag="fold")
                        nc.vector.tensor_tensor(
                            fold[:], s16[:, 0:1024], s16[:, 1024:2048], MIN
                        )
                        nc.vector.tensor_tensor(
                            fold[:, 0:512], fold[:, 0:512], fold[:, 512:1024], MIN
                        )
                        nc.vector.tensor_reduce(
                            out=colsb[:, b * MCH + i : b * MCH + i + 1],
                            in_=fold[:, 0:512],
                            axis=mybir.AxisListType.X,
                            op=MIN,
                        )
                        sl = racc[:, b, :]
                        if i == 0:
                            nc.vector.tensor_copy(sl, s16[:])
                        else:
                            nc.vector.tensor_tensor(sl, s16[:], sl, MIN)
            nc.sync.dma_start(out=out16[:], in_=packed[:])

    _fix_walrus_waits(nc)
    return nc


def _prep_core_inputs(x, y, c):
    """fp16 hi/lo compensated K=15 feature stacks for this core's batches."""
    xb = x[BPC * c : BPC * (c + 1)].astype(np.float32)  # [4, 2048, 3]
    yb = y[BPC * c : BPC * (c + 1)].astype(np.float32)
    ones = np.ones((BPC, N), np.float32)
    x2 = np.sum(xb**2, axis=-1)  # [4, N]
    y2 = np.sum(yb**2, axis=-1)  # [4, M]
    X = np.stack(
        [-2.0 * xb[..., 0], -2.0 * xb[..., 1], -2.0 * xb[..., 2], ones, x2], axis=1
    )  # [4, 5, N]
    Y = np.stack([yb[..., 0], yb[..., 1], yb[..., 2], y2, ones], axis=1)
    Xh = X.astype(np.float16)
    Xl = (X - Xh.astype(np.float32)).astype(np.float16)
    Yh = Y.astype(np.float16)
    Yl = (Y - Yh.astype(np.float32)).astype(np.float16)
    rhs = np.concatenate([Xh, Xl, Xh], axis=1)  # [4, 15, N]
    lhs = np.concatenate([Yh, Yh, Yl], axis=1)  # [4, 15, M]
    return np.ascontiguousarray(np.stack([rhs, lhs], axis=0))  # [2, 4, 15, N]


def kernel(x, y):
    global LAST_RESULTS
    from concourse.bass_utils import run_bass_kernel_spmd

    x = np.asarray(x, dtype=np.float32)
    y = np.asarray(y, dtype=np.float32)
    assert x.shape == (B, N, D) and y.shape == (B, M, D)

    if "nc" not in _CACHE:
        _CACHE["nc"] = _build_bass()
    nc = _CACHE["nc"]

    in_maps = []
    for c in range(N_CORES):
        in_maps.append({"feats": _prep_core_inputs(x, y, c)})

    res = run_bass_kernel_spmd(nc, in_maps, core_ids=list(range(N_CORES)))
    LAST_RESULTS = res

    cham = np.zeros((B,), np.float64)
    for c in range(N_CORES):
        out = res.results[c]["out16"]  # [128, BPC*N + BPC*MCH] fp16
        rowacc = out[:, : BPC * N].reshape(128, BPC, N)
        colmin = out[:, BPC * N :].reshape(128, BPC, MCH)
        rowmin = rowacc.min(axis=0).astype(np.float64)  # [4, N]
        row = rowmin.mean(axis=1)  # [4]
        for b in range(BPC):
            col = colmin[:, b, :].astype(np.float64).mean()
            cham[BPC * c + b] = max(row[b], col)
    return np.float32(cham.mean())
